# revision 9
# baseline (speedup 1.0000x reference)
"""Multi-head graph attention layer (GAT) on 8 TRN2 NeuronCores.

Row-parallel sharding: core c owns destination rows [c*512, (c+1)*512).
Scores are materialized transposed (source j on partitions, dest i on free dim)
so that alpha @ Wx is a single accumulating matmul per (j-chunk, head) with the
softmax denominator obtained from an appended ones-column in lhsT.

Score math: exp(leakyrelu(s)) with s = a_src[i] + a_dst[j] factors as
    exp(prelu(s)) = exp(0.2 s) * max(exp(0.8 s), 1)
The rank-1 factor exp(0.2 s) = exp(.2 a_src[i]) * exp(.2 a_dst[j]) needs no
elementwise work: the i-part is constant per softmax row and cancels between
numerator and denominator; the j-part folds into the matmul weights
(L2 = exp(.2 a_dst[j]) * [Wx | 1], built once in phase A). Per j-chunk the
loop computes  un' = max(exp(.8 s), 1) * mask  via one of two paths that
differ only in which engine produces exp(.8 s):
  PE+ACT path: rank-2 matmul builds .8 s in PSUM (hi/lo bf16 split keeps
               f32-ish accuracy), one batched Exp (PSUM->SBUF);
  DVE path:    exp(.8 s) = R8[i]*G8[j] as 4 tensor_scalar mults (4x mode).
Both finish with ONE batched scalar_tensor_tensor: (u max 1) * mask, where
mask broadcasts across heads with a stride-0 AP.
"""

import os
import numpy as np
import ml_dtypes

import concourse.bacc as bacc
import concourse.mybir as mybir
import concourse.tile as tile
from concourse.bass_utils import run_bass_kernel_spmd
from concourse.masks import make_identity

N, Q, D, H = 4096, 512, 64, 4
NCORES = 8
S = N // NCORES          # 512 dest rows per core
NJ = N // 128            # 32 j-chunks
NI = S // 128            # 4 i-chunks per core
NQ = Q // 128            # 4 q-chunks
NEG = 0.2
POS = 1.0 - NEG          # 0.8
LN_EPS = 1e-5
ACT_JC = int(os.environ.get("ACT_JC", "21"))   # of 32 jc groups on the PE+ACT path
GPS_JC = int(os.environ.get("GPS_JC", "0"))    # of 32: final stt on gpsimd
GPSC = bool(int(os.environ.get("GPSC", "1")))  # phase-C TTs on gpsimd
REPEAT = int(os.environ.get("REPEAT", "1"))    # repeat main loop (timing amplification)
TPOOL_B = int(os.environ.get("TPOOL_B", "3"))
MPOOL_B = int(os.environ.get("MPOOL_B", "12"))
f32 = mybir.dt.float32
bf16 = mybir.dt.bfloat16
AF = mybir.ActivationFunctionType
ALU = mybir.AluOpType

_NC_CACHE = {}


def _build():
    nc = bacc.Bacc("TRN2", target_bir_lowering=False)

    xt = nc.declare_dram_parameter("xt", [Q, N], bf16, isOutput=False)
    xst = nc.declare_dram_parameter("xst", [Q, S], bf16, isOutput=False)
    mbt = nc.declare_dram_parameter("mbt", [N, S], bf16, isOutput=False)
    wp = nc.declare_dram_parameter("wp", [NQ, 128, H, 66], bf16, isOutput=False)
    gb = nc.declare_dram_parameter("gb", [128, 2, 256], f32, isOutput=False)
    out = nc.declare_dram_parameter("out", [S, 256], f32, isOutput=True)

    with tile.TileContext(nc) as tc:
        with (
            tc.tile_pool(name="consts", bufs=1) as consts,
            tc.tile_pool(name="mpool", bufs=MPOOL_B) as mpool,
            tc.tile_pool(name="tpool", bufs=TPOOL_B) as tpool,
            tc.tile_pool(name="fpool", bufs=4) as fpool,
            tc.tile_pool(name="ppool", bufs=1, space="PSUM") as ppool,
        ):
            def ctile(shape, dtype, tg):
                return consts.tile(shape, dtype, tag=tg, name=tg)

            def ptile(name):
                # 2-bank PSUM scratch (tag-rotated, 2 deep); phase A/C carve
                # views out of it, the loop uses it for score tiles
                return ppool.tile([128, 2, 512], f32, tag="sc", name=name, bufs=2)

            # ---------------- constants / small inputs ----------------
            wp_sb = ctile([128, NQ, H, 66], bf16, "wp_sb")
            nc.scalar.dma_start(out=wp_sb, in_=wp.rearrange("qc p h d -> p qc h d"))
            gb_sb = ctile([128, 2, 256], f32, "gb_sb")
            nc.scalar.dma_start(out=gb_sb, in_=gb[:, :, :])
            ident = ctile([128, 128], f32, "ident")
            make_identity(nc, ident)
            identb = ctile([128, 128], bf16, "identb")
            nc.vector.tensor_copy(identb, ident)

            eps_t = ctile([128, 1], f32, "eps_t")
            nc.vector.memset(eps_t, LN_EPS)

            # ---------------- xT loads (host pre-transposed) ----------------
            xsT_sb = ctile([128, NQ, S], bf16, "xsT_sb")
            nc.scalar.dma_start(out=xsT_sb, in_=xst.rearrange("(qc p) n -> p qc n", p=128))
            xt_sb = ctile([128, NQ, N], bf16, "xt_sb")
            for ch in range(8):
                n0, n1 = ch * (N // 8), (ch + 1) * (N // 8)
                nc.sync.dma_start(
                    out=xt_sb[:, :, n0:n1],
                    in_=xt[:, n0:n1].rearrange("(qc p) n -> p qc n", p=128),
                )

            # ---------------- phase A: Wx' = x @ [W | w_src | w_dst] ----------------
            # L2_sb[:, jc, h, 0:64] = exp(.2 a_dst[j]) * Wx (bf16),
            # col 64 = exp(.2 a_dst[j])  (denominator column)
            L2_sb = ctile([128, NJ, H, 65], bf16, "L2_sb")
            F2 = ctile([128, NJ, H], f32, "F2")     # exp(.2 a_dst)
            G8 = ctile([128, NJ, H], f32, "G8")     # exp(.8 a_dst)
            G4 = ctile([128, NJ, H], f32, "G4")
            ad_sb = ctile([128, NJ, H, 2], f32, "ad_sb")  # [...,0]=a_src(n) [...,1]=a_dst(n)
            for nc_ in range(NJ):
                pwt = ptile(f"pw{nc_}")
                pw = pwt.rearrange("p a b -> p (a b)")[:, 0:H * 66].rearrange(
                    "p (h d) -> p h d", d=66)
                for qc in range(NQ):
                    nc.tensor.matmul(
                        pw, xt_sb[:, qc, nc_ * 128:(nc_ + 1) * 128], wp_sb[:, qc, :, :],
                        start=(qc == 0), stop=(qc == NQ - 1),
                    )
                nc.vector.tensor_copy(ad_sb[:, nc_, :, :], pw[:, :, 64:66])
                nc.scalar.activation(out=F2[:, nc_, :], in_=ad_sb[:, nc_, :, 1],
                                     func=AF.Exp, scale=NEG)
                for h in range(H):
                    if h % 2 == 0:
                        nc.vector.tensor_scalar(
                            out=L2_sb[:, nc_, h, 0:64], in0=pw[:, h, 0:64],
                            scalar1=F2[:, nc_, h:h + 1], scalar2=None, op0=ALU.mult,
                        )
                    else:
                        nc.scalar.activation(
                            out=L2_sb[:, nc_, h, 0:64], in_=pw[:, h, 0:64],
                            func=AF.Copy, scale=F2[:, nc_, h:h + 1],
                        )
            # denominator column: L2[:, :, :, 64] = F2
            nc.vector.tensor_copy(L2_sb[:, :, :, 64], F2)
            # G8 = F2^4 = exp(.8 a_dst)
            nc.scalar.activation(out=G4, in_=F2, func=AF.Square)
            nc.scalar.activation(out=G8, in_=G4, func=AF.Square)

            # ad8 = .8 a_dst, split hi/lo bf16 and transposed into lhsT rows for
            # the rank-2 score matmul (adsT[0]=hi, [1]=lo, [2:4]=ones)
            ad8 = ctile([128, NJ, H], f32, "ad8")
            nc.vector.tensor_scalar(out=ad8, in0=ad_sb[:, :, :, 1], scalar1=POS,
                                    scalar2=None, op0=ALU.mult)
            adsT_sb = ctile([4, NJ, H, 128], bf16, "adsT_sb")
            nc.vector.memset(adsT_sb, 1.0)   # rows 2:4 stay ones
            pt = ptile("ptr")
            ptv = pt.rearrange("p a b -> p (a b)")[:, 0:128]
            nc.tensor.transpose(ptv, ad8.rearrange("p a b -> p (a b)"), ident)
            adT_hi = ctile([128, 128], bf16, "adT_hi")
            nc.vector.tensor_copy(adT_hi, ptv)
            adT_lo = ctile([128, 128], bf16, "adT_lo")
            nc.vector.tensor_tensor(out=adT_lo, in0=ptv, in1=adT_hi, op=ALU.subtract)
            nc.sync.dma_start(out=adsT_sb[0:1], in_=adT_hi)
            nc.sync.dma_start(out=adsT_sb[1:2], in_=adT_lo)

            # ---------------- a_src rows for this core's shard ----------------
            pat = ptile("p_asrc")
            p_asrc = pat.rearrange("p a b -> p (a b)")[:, 0:512]
            for qc in range(NQ):
                nc.tensor.matmul(
                    p_asrc[0:H, :], wp_sb[:, qc, :, 64], xsT_sb[:, qc, :],
                    start=(qc == 0), stop=(qc == NQ - 1),
                )
            asrc_row = ctile([H, S], f32, "asrc_row")
            nc.vector.tensor_copy(asrc_row, p_asrc[0:H, :])
            r8_row = ctile([H, S], bf16, "r8_row")
            nc.scalar.activation(out=r8_row, in_=asrc_row, func=AF.Exp, scale=POS)
            # rhs rows for the rank-2 score matmul: [1, 1, as_hi, as_lo]
            as8 = ctile([H, S], f32, "as8")
            nc.vector.tensor_scalar(out=as8, in0=asrc_row, scalar1=POS, scalar2=None,
                                    op0=ALU.mult)
            as_hi = ctile([H, S], bf16, "as_hi")
            nc.vector.tensor_copy(as_hi, as8)
            as_lo = ctile([H, S], bf16, "as_lo")
            nc.vector.tensor_tensor(out=as_lo, in0=as8, in1=as_hi, op=ALU.subtract)
            rhs8_sb = ctile([4, H, S], bf16, "rhs8_sb")
            nc.vector.memset(rhs8_sb[0:2], 1.0)
            nc.sync.dma_start(out=rhs8_sb[2:3], in_=as_hi)
            nc.sync.dma_start(out=rhs8_sb[3:4], in_=as_lo)

            # broadcast exp(.8 a_src) row h across partitions via selector matmul
            iota_p128 = ctile([128, 128], f32, "iota_p128")
            nc.gpsimd.iota(iota_p128, pattern=[[0, 128]], base=0, channel_multiplier=1,
                           allow_small_or_imprecise_dtypes=True)
            sel_tb = ctile([128, H, 128], bf16, "sel_tb")
            for h in range(H):
                nc.vector.tensor_scalar(
                    out=sel_tb[:, h, :], in0=iota_p128, scalar1=float(h), scalar2=None,
                    op0=ALU.is_equal,
                )
            R8b = ctile([128, H, S], bf16, "R8b")        # exp(.8 a_src) broadcast
            for h in range(H):
                pbt = ptile(f"pb_r8{h}")
                pb = pbt.rearrange("p a b -> p (a b)")[:, 0:512]
                nc.tensor.matmul(pb, sel_tb[0:H, h, :], r8_row, start=True, stop=True)
                nc.scalar.copy(R8b[:, h, :], pb)

            # ---------------- phase B: attention main loop ----------------
            # psum accumulators, one [65, 512] bank per head:
            # rows 0:64 = outT'[d, i] (unnormalized); row 64 = S'[i] (denominator)
            poT = [ppool.tile([65, 512], f32, tag=f"oT{h}", name=f"oT{h}")
                   for h in range(H)]

            import contextlib
            loop_cm = tc.For_i(0, REPEAT, 1) if REPEAT > 1 else contextlib.nullcontext()
            with loop_cm:
              rep = 0
              for jc in range(NJ):
                mT = mpool.tile([128, S], bf16, tag="mask", name=f"mT{rep}_{jc}")
                nc.sync.dma_start(out=mT, in_=mbt[jc * 128:(jc + 1) * 128, :])

                if (jc * ACT_JC) % NJ < ACT_JC:
                    # PE+ACT path: sc = .8 s via rank-2 matmul, u = Exp(sc)
                    src = tpool.tile([128, H, S], bf16, tag="u", name=f"u{rep}_{jc}")
                    for hp in range(2):
                        sct = ptile(f"sc{rep}_{jc}_{hp}")
                        for k in range(2):
                            h = hp * 2 + k
                            nc.tensor.matmul(
                                sct[:, k, :], adsT_sb[:, jc, h, :], rhs8_sb[:, h, :],
                                start=True, stop=True,
                            )
                        nc.scalar.activation(out=src[:, hp * 2:hp * 2 + 2, :], in_=sct,
                                             func=AF.Exp)
                else:
                    # DVE path: u = R8[i] * G8[j]
                    src = tpool.tile([128, H, S], bf16, tag="m", name=f"m{rep}_{jc}")
                    for h in range(H):
                        nc.vector.tensor_scalar(
                            out=src[:, h, :], in0=R8b[:, h, :],
                            scalar1=G8[:, jc, h:h + 1], scalar2=None, op0=ALU.mult,
                        )

                # un = max(u, 1) * mask  (single fused op, mask broadcast over h)
                un = tpool.tile([128, H, S], bf16, tag="un", name=f"un{rep}_{jc}")
                mTb = mT.unsqueeze(1).broadcast_to([128, H, S])
                eng = nc.gpsimd if (jc * GPS_JC) % NJ < GPS_JC else nc.vector
                eng.scalar_tensor_tensor(out=un, in0=src, scalar=1.0, in1=mTb,
                                         op0=ALU.max, op1=ALU.mult)

                for h in range(H):
                    nc.tensor.matmul(
                        poT[h], L2_sb[:, jc, h, 0:65], un[:, h, :],
                        start=(jc == 0), stop=(jc == NJ - 1),
                    )

            # ---------------- phase C: normalize, ELU, LayerNorm ----------------
            oT_sb = ctile([65, H, S], f32, "oT_sb")
            for h in range(H):
                if h % 2 == 0:
                    nc.vector.tensor_copy(oT_sb[:, h, :], poT[h])
                else:
                    nc.scalar.copy(oT_sb[:, h, :], poT[h])

            for ic in range(NI):
                p2t = ptile(f"p2_{ic}")
                p2 = p2t.rearrange("p a b -> p (a b)")[:, 0:H * 66].rearrange(
                    "p (h d) -> p h d", d=66)
                for h in range(H):
                    nc.tensor.transpose(
                        p2[:, h, 0:65],
                        oT_sb[:, h, ic * 128:(ic + 1) * 128],
                        ident[0:65, 0:65],
                    )
                s_sb = fpool.tile([128, H], f32, tag="s", name=f"s{ic}")
                nc.vector.tensor_copy(s_sb, p2[:, :, 64])
                rs = fpool.tile([128, H], f32, tag="rs", name=f"rs{ic}")
                nc.vector.reciprocal(rs, s_sb)

                o = fpool.tile([128, 256], f32, tag="o", name=f"o{ic}")
                ov = o.rearrange("p (h d) -> p h d", h=H)
                for h in range(H):
                    nc.vector.tensor_scalar(
                        out=ov[:, h, :], in0=p2[:, h, 0:64], scalar1=rs[:, h:h + 1],
                        scalar2=None, op0=ALU.mult,
                    )
                # ELU: exp(min(o,0)) + max(o,0) - 1
                m1 = fpool.tile([128, 256], f32, tag="m1", name=f"m1_{ic}")
                nc.vector.tensor_scalar(out=m1, in0=o, scalar1=0.0, scalar2=None, op0=ALU.min)
                e1 = fpool.tile([128, 256], f32, tag="e1", name=f"e1_{ic}")
                nc.scalar.activation(out=e1, in_=m1, func=AF.Exp)
                r1 = fpool.tile([128, 256], f32, tag="r1", name=f"r1_{ic}")
                nc.vector.tensor_scalar(out=r1, in0=o, scalar1=0.0, scalar2=None, op0=ALU.max)
                (nc.gpsimd if GPSC else nc.vector).tensor_tensor(out=e1, in0=e1, in1=r1, op=ALU.add)
                nc.vector.tensor_scalar(out=e1, in0=e1, scalar1=1.0, scalar2=None,
                                        op0=ALU.subtract)

                # LayerNorm over 256 features
                st6 = fpool.tile([128, 6], f32, tag="st6", name=f"st6_{ic}")
                nc.vector.bn_stats(out=st6, in_=e1)
                mv = fpool.tile([128, 2], f32, tag="mv", name=f"mv{ic}")
                nc.vector.bn_aggr(out=mv, in_=st6)
                sd = fpool.tile([128, 1], f32, tag="sd", name=f"sd{ic}")
                nc.scalar.activation(out=sd, in_=mv[:, 1:2], func=AF.Sqrt, bias=eps_t)
                rstd = fpool.tile([128, 1], f32, tag="rstd", name=f"rstd{ic}")
                nc.vector.reciprocal(rstd, sd)
                xm = fpool.tile([128, 256], f32, tag="xm", name=f"xm{ic}")
                nc.vector.tensor_scalar(
                    out=xm, in0=e1, scalar1=mv[:, 0:1], scalar2=rstd,
                    op0=ALU.subtract, op1=ALU.mult,
                )
                (nc.gpsimd if GPSC else nc.vector).tensor_tensor(out=xm, in0=xm, in1=gb_sb[:, 0, :], op=ALU.mult)
                (nc.gpsimd if GPSC else nc.vector).tensor_tensor(out=xm, in0=xm, in1=gb_sb[:, 1, :], op=ALU.add)
                nc.scalar.dma_start(out=out[ic * 128:(ic + 1) * 128, :], in_=xm)

    nc.compile()
    return nc


def _prep_in_maps(x, adj, W, a, gamma, beta):
    x = np.asarray(x)
    adj = np.asarray(adj)
    W = np.asarray(W, np.float32)
    a = np.asarray(a, np.float32)
    gamma = np.asarray(gamma, np.float32)
    beta = np.asarray(beta, np.float32)

    # weight folding (host): w_src = W @ a[:, :D], w_dst = W @ a[:, D:]
    w_src = np.einsum("hqd,hd->hq", W, a[:, :D]).astype(np.float32)   # (H, Q)
    w_dst = np.einsum("hqd,hd->hq", W, a[:, D:]).astype(np.float32)   # (H, Q)
    Wp = np.concatenate([W, w_src[:, :, None], w_dst[:, :, None]], axis=2)  # (H, Q, 66)
    wp_in = np.ascontiguousarray(
        Wp.transpose(1, 0, 2).reshape(NQ, 128, H, 66)
    ).astype(ml_dtypes.bfloat16)

    xb = x.astype(ml_dtypes.bfloat16)
    xtb = np.ascontiguousarray(xb.T)                      # (Q, N)
    mbf = (adj > 0).astype(ml_dtypes.bfloat16)
    np.fill_diagonal(mbf, np.float32(1.0))
    mbt_full = np.ascontiguousarray(mbf.T)                # (N, N): mbt_full[j, i]
    gb_in = np.broadcast_to(
        np.stack([gamma, beta])[None, :, :], (128, 2, 256)
    ).astype(np.float32).copy()

    in_maps = []
    for c in range(NCORES):
        off = c * S
        in_maps.append({
            "xt": xtb,
            "xst": np.ascontiguousarray(xtb[:, off:off + S]),
            "mbt": np.ascontiguousarray(mbt_full[:, off:off + S]),
            "wp": wp_in,
            "gb": gb_in,
        })
    return in_maps


def kernel(x, adj, W, a, gamma, beta):
    in_maps = _prep_in_maps(x, adj, W, a, gamma, beta)

    key = ("gat", REPEAT, ACT_JC, GPS_JC, GPSC)
    if key not in _NC_CACHE:
        _NC_CACHE[key] = _build()
    nc = _NC_CACHE[key]

    trace = bool(int(os.environ.get("KERNEL_TRACE", "0")))
    try:
        import antenv.axon_hooks  # noqa: F401
    except Exception:
        trace = False
    res = run_bass_kernel_spmd(nc, in_maps, core_ids=list(range(NCORES)), trace=trace)
    if trace and res.exec_time_ns is not None:
        print(f"HW exec time: {res.exec_time_ns} ns")
        print(f"mean exec time: {res.mean_exec_time_ns} ns")
        if res.instructions_and_trace is not None:
            print("trace:", res.instructions_and_trace[1])
    return np.concatenate([res.results[c]["out"] for c in range(NCORES)], axis=0)


# revision 15
# speedup vs baseline: 1.5622x; 1.5622x over previous
"""Multi-head graph attention layer (GAT) on 8 TRN2 NeuronCores.

Row-parallel sharding: core c owns destination rows [c*512, (c+1)*512).
Scores are materialized transposed (source j on partitions, dest i on free dim)
so that alpha @ Wx is a single accumulating matmul per (j-chunk, head) with the
softmax denominator obtained from an appended ones-column in lhsT.

Score math: exp(leakyrelu(s)) with s = a_src[i] + a_dst[j] factors as
    exp(prelu(s)) = exp(0.2 s) * max(exp(0.8 s), 1)
The rank-1 factor exp(0.2 s) = exp(.2 a_src[i]) * exp(.2 a_dst[j]) needs no
elementwise work: the i-part is constant per softmax row and cancels between
numerator and denominator; the j-part folds into the matmul weights
(L2 = exp(.2 a_dst[j]) * [Wx | 1], built once in phase A). Per j-chunk the
loop computes  un' = max(exp(.8 s), 1) * mask  via one of two paths that
differ only in which engine produces exp(.8 s):
  PE+ACT path: rank-2 matmul builds .8 s in PSUM (hi/lo bf16 split keeps
               f32-ish accuracy), one batched Exp (PSUM->SBUF);
  DVE path:    exp(.8 s) = R8[i]*G8[j] as 4 tensor_scalar mults (4x mode).
Both finish with ONE batched scalar_tensor_tensor: (u max 1) * mask, where
mask broadcasts across heads with a stride-0 AP.
"""

import os
import numpy as np
import ml_dtypes

import concourse.bacc as bacc
import concourse.mybir as mybir
import concourse.tile as tile
from concourse.bass_utils import run_bass_kernel_spmd
from concourse.masks import make_identity

N, Q, D, H = 4096, 512, 64, 4
NCORES = 8
S = N // NCORES          # 512 dest rows per core
NJ = N // 128            # 32 j-chunks
NI = S // 128            # 4 i-chunks per core
NQ = Q // 128            # 4 q-chunks
NEG = 0.2
POS = 1.0 - NEG          # 0.8
LN_EPS = 1e-5
ACT_JC = int(os.environ.get("ACT_JC", "21"))   # of 32 jc groups on the PE+ACT path
GPS_JC = int(os.environ.get("GPS_JC", "0"))    # of 32: final stt on gpsimd
GPSC = bool(int(os.environ.get("GPSC", "1")))  # phase-C TTs on gpsimd
REPEAT = int(os.environ.get("REPEAT", "1"))    # repeat main loop (timing amplification)
TPOOL_B = int(os.environ.get("TPOOL_B", "3"))
MPOOL_B = int(os.environ.get("MPOOL_B", "12"))
f32 = mybir.dt.float32
bf16 = mybir.dt.bfloat16
AF = mybir.ActivationFunctionType
ALU = mybir.AluOpType

_NC_CACHE = {}


def _build():
    nc = bacc.Bacc("TRN2", target_bir_lowering=False)

    xt = nc.declare_dram_parameter("xt", [Q, N], bf16, isOutput=False)
    xst = nc.declare_dram_parameter("xst", [Q, S], bf16, isOutput=False)
    mbt = nc.declare_dram_parameter("mbt", [N, S], bf16, isOutput=False)
    wp = nc.declare_dram_parameter("wp", [NQ, 128, H, 66], bf16, isOutput=False)
    gb = nc.declare_dram_parameter("gb", [128, 2, 256], f32, isOutput=False)
    out = nc.declare_dram_parameter("out", [S, 256], f32, isOutput=True)

    with tile.TileContext(nc) as tc:
        with (
            tc.tile_pool(name="consts", bufs=1) as consts,
            tc.tile_pool(name="mpool", bufs=MPOOL_B) as mpool,
            tc.tile_pool(name="tpool", bufs=TPOOL_B) as tpool,
            tc.tile_pool(name="fpool", bufs=4) as fpool,
            tc.tile_pool(name="ppool", bufs=1, space="PSUM") as ppool,
        ):
            def ctile(shape, dtype, tg):
                return consts.tile(shape, dtype, tag=tg, name=tg)

            def ptile(name):
                # 2-bank PSUM scratch (tag-rotated, 2 deep); phase A/C carve
                # views out of it, the loop uses it for score tiles
                return ppool.tile([128, 2, 512], f32, tag="sc", name=name, bufs=2)

            # ---------------- constants / small inputs ----------------
            wp_sb = ctile([128, NQ, H, 66], bf16, "wp_sb")
            nc.scalar.dma_start(out=wp_sb, in_=wp.rearrange("qc p h d -> p qc h d"))
            gb_sb = ctile([128, 2, 256], f32, "gb_sb")
            nc.scalar.dma_start(out=gb_sb, in_=gb[:, :, :])
            ident = ctile([128, 128], f32, "ident")
            make_identity(nc, ident)

            eps_t = ctile([128, 1], f32, "eps_t")
            nc.vector.memset(eps_t, LN_EPS)

            # ---------------- xT loads (host pre-transposed) ----------------
            xsT_sb = ctile([128, NQ, S], bf16, "xsT_sb")
            nc.scalar.dma_start(out=xsT_sb, in_=xst.rearrange("(qc p) n -> p qc n", p=128))
            xt_sb = ctile([128, NQ, N], bf16, "xt_sb")
            for ch in range(8):
                n0, n1 = ch * (N // 8), (ch + 1) * (N // 8)
                nc.sync.dma_start(
                    out=xt_sb[:, :, n0:n1],
                    in_=xt[:, n0:n1].rearrange("(qc p) n -> p qc n", p=128),
                )

            # ---------------- phase A: Wx' = x @ [W | w_src | w_dst] ----------------
            # L2_sb[:, jc, h, 0:64] = exp(.2 a_dst[j]) * Wx (bf16),
            # col 64 = exp(.2 a_dst[j])  (denominator column)
            L2_sb = ctile([128, NJ, H, 65], bf16, "L2_sb")
            F2 = ctile([128, NJ, H], f32, "F2")     # exp(.2 a_dst)
            G8 = ctile([128, NJ, H], f32, "G8")     # exp(.8 a_dst)
            G4 = ctile([128, NJ, H], f32, "G4")
            ad_sb = ctile([128, NJ, H, 2], f32, "ad_sb")  # [...,0]=a_src(n) [...,1]=a_dst(n)
            for nc_ in range(NJ):
                pwt = ptile(f"pw{nc_}")
                pw = pwt.rearrange("p a b -> p (a b)")[:, 0:H * 66].rearrange(
                    "p (h d) -> p h d", d=66)
                for qc in range(NQ):
                    nc.tensor.matmul(
                        pw, xt_sb[:, qc, nc_ * 128:(nc_ + 1) * 128], wp_sb[:, qc, :, :],
                        start=(qc == 0), stop=(qc == NQ - 1),
                    )
                nc.vector.tensor_copy(ad_sb[:, nc_, :, :], pw[:, :, 64:66])
                nc.scalar.activation(out=F2[:, nc_, :], in_=ad_sb[:, nc_, :, 1],
                                     func=AF.Exp, scale=NEG)
                for h in range(H):
                    if h % 2 == 0:
                        nc.vector.tensor_scalar(
                            out=L2_sb[:, nc_, h, 0:64], in0=pw[:, h, 0:64],
                            scalar1=F2[:, nc_, h:h + 1], scalar2=None, op0=ALU.mult,
                        )
                    else:
                        nc.scalar.activation(
                            out=L2_sb[:, nc_, h, 0:64], in_=pw[:, h, 0:64],
                            func=AF.Copy, scale=F2[:, nc_, h:h + 1],
                        )
            # denominator column: L2[:, :, :, 64] = F2
            nc.vector.tensor_copy(L2_sb[:, :, :, 64], F2)
            # G8 = F2^4 = exp(.8 a_dst)
            nc.scalar.activation(out=G4, in_=F2, func=AF.Square)
            nc.scalar.activation(out=G8, in_=G4, func=AF.Square)

            # ad8 = .8 a_dst (per-partition bias rows for the ACT path)
            ad8 = ctile([128, NJ, H], f32, "ad8")
            nc.vector.tensor_scalar(out=ad8, in0=ad_sb[:, :, :, 1], scalar1=POS,
                                    scalar2=None, op0=ALU.mult)

            # ---------------- a_src rows for this core's shard ----------------
            pat = ptile("p_asrc")
            p_asrc = pat.rearrange("p a b -> p (a b)")[:, 0:512]
            for qc in range(NQ):
                nc.tensor.matmul(
                    p_asrc[0:H, :], wp_sb[:, qc, :, 64], xsT_sb[:, qc, :],
                    start=(qc == 0), stop=(qc == NQ - 1),
                )
            asrc_row = ctile([H, S], f32, "asrc_row")
            nc.vector.tensor_copy(asrc_row, p_asrc[0:H, :])
            r8_row = ctile([H, S], bf16, "r8_row")
            nc.scalar.activation(out=r8_row, in_=asrc_row, func=AF.Exp, scale=POS)

            # broadcast a_src / exp(.8 a_src) row h across partitions via
            # selector matmuls (sel[:, h, :] has ones on partition h only)
            iota_p128 = ctile([128, 128], f32, "iota_p128")
            nc.gpsimd.iota(iota_p128, pattern=[[0, 128]], base=0, channel_multiplier=1,
                           allow_small_or_imprecise_dtypes=True)
            sel_t = ctile([128, H, 128], f32, "sel_t")
            sel_tb = ctile([128, H, 128], bf16, "sel_tb")
            for h in range(H):
                nc.vector.tensor_scalar(
                    out=sel_t[:, h, :], in0=iota_p128, scalar1=float(h), scalar2=None,
                    op0=ALU.is_equal,
                )
                nc.vector.tensor_scalar(
                    out=sel_tb[:, h, :], in0=iota_p128, scalar1=float(h), scalar2=None,
                    op0=ALU.is_equal,
                )
            asrc_b = ctile([128, H, S], f32, "asrc_b")   # a_src broadcast (ACT path)
            R8b = ctile([128, H, S], bf16, "R8b")        # exp(.8 a_src) broadcast
            for h in range(H):
                pbt = ptile(f"pb_a{h}")
                pb = pbt.rearrange("p a b -> p (a b)")[:, 0:512]
                nc.tensor.matmul(pb, sel_t[0:H, h, :], asrc_row, start=True, stop=True)
                nc.vector.tensor_copy(asrc_b[:, h, :], pb)
                pbt = ptile(f"pb_r8{h}")
                pb = pbt.rearrange("p a b -> p (a b)")[:, 0:512]
                nc.tensor.matmul(pb, sel_tb[0:H, h, :], r8_row, start=True, stop=True)
                nc.scalar.copy(R8b[:, h, :], pb)

            # ---------------- phase B: attention main loop ----------------
            # psum accumulators, one [65, 512] bank per head:
            # rows 0:64 = outT'[d, i] (unnormalized); row 64 = S'[i] (denominator)
            poT = [ppool.tile([65, 512], f32, tag=f"oT{h}", name=f"oT{h}")
                   for h in range(H)]

            import contextlib
            loop_cm = tc.For_i(0, REPEAT, 1) if REPEAT > 1 else contextlib.nullcontext()
            with loop_cm:
              rep = 0
              for jc in range(NJ):
                mT = mpool.tile([128, S], bf16, tag="mask", name=f"mT{rep}_{jc}")
                nc.sync.dma_start(out=mT, in_=mbt[jc * 128:(jc + 1) * 128, :])

                if (jc * ACT_JC) % NJ < ACT_JC:
                    # ACT path: t = Relu(.8 s), u = Exp(t) = max(exp(.8 s), 1)
                    t = tpool.tile([128, H, S], f32, tag="t", name=f"t{rep}_{jc}",
                                   bufs=2)
                    for h in range(H):
                        nc.scalar.activation(
                            out=t[:, h, :], in_=asrc_b[:, h, :], func=AF.Relu,
                            bias=ad8[:, jc, h:h + 1], scale=POS,
                        )
                    src = tpool.tile([128, H, S], bf16, tag="u", name=f"u{rep}_{jc}")
                    nc.scalar.activation(out=src, in_=t, func=AF.Exp)
                else:
                    # DVE path: u = max(R8[i] * G8[j], 1)
                    src = tpool.tile([128, H, S], bf16, tag="m", name=f"m{rep}_{jc}")
                    for h in range(H):
                        nc.vector.tensor_scalar(
                            out=src[:, h, :], in0=R8b[:, h, :],
                            scalar1=G8[:, jc, h:h + 1], scalar2=1.0,
                            op0=ALU.mult, op1=ALU.max,
                        )

                # un = u * mask  (mask broadcast over h with a stride-0 AP)
                un = tpool.tile([128, H, S], bf16, tag="un", name=f"un{rep}_{jc}")
                mTb = mT.unsqueeze(1).broadcast_to([128, H, S])
                eng = nc.gpsimd if (jc * GPS_JC) % NJ < GPS_JC else nc.vector
                eng.tensor_tensor(out=un, in0=src, in1=mTb, op=ALU.mult)

                for h in range(H):
                    nc.tensor.matmul(
                        poT[h], L2_sb[:, jc, h, 0:65], un[:, h, :],
                        start=(jc == 0), stop=(jc == NJ - 1),
                    )

            # ---------------- phase C: normalize, ELU, LayerNorm ----------------
            oT_sb = ctile([65, H, S], f32, "oT_sb")
            for h in range(H):
                if h % 2 == 0:
                    nc.vector.tensor_copy(oT_sb[:, h, :], poT[h])
                else:
                    nc.scalar.copy(oT_sb[:, h, :], poT[h])

            for ic in range(NI):
                p2t = ptile(f"p2_{ic}")
                p2 = p2t.rearrange("p a b -> p (a b)")[:, 0:H * 66].rearrange(
                    "p (h d) -> p h d", d=66)
                for h in range(H):
                    nc.tensor.transpose(
                        p2[:, h, 0:65],
                        oT_sb[:, h, ic * 128:(ic + 1) * 128],
                        ident[0:65, 0:65],
                    )
                s_sb = fpool.tile([128, H], f32, tag="s", name=f"s{ic}")
                nc.vector.tensor_copy(s_sb, p2[:, :, 64])
                rs = fpool.tile([128, H], f32, tag="rs", name=f"rs{ic}")
                nc.vector.reciprocal(rs, s_sb)

                o = fpool.tile([128, 256], f32, tag="o", name=f"o{ic}")
                ov = o.rearrange("p (h d) -> p h d", h=H)
                for h in range(H):
                    nc.vector.tensor_scalar(
                        out=ov[:, h, :], in0=p2[:, h, 0:64], scalar1=rs[:, h:h + 1],
                        scalar2=None, op0=ALU.mult,
                    )
                # ELU: exp(min(o,0)) + max(o,0) - 1
                m1 = fpool.tile([128, 256], f32, tag="m1", name=f"m1_{ic}")
                nc.vector.tensor_scalar(out=m1, in0=o, scalar1=0.0, scalar2=None, op0=ALU.min)
                e1 = fpool.tile([128, 256], f32, tag="e1", name=f"e1_{ic}")
                nc.scalar.activation(out=e1, in_=m1, func=AF.Exp)
                r1 = fpool.tile([128, 256], f32, tag="r1", name=f"r1_{ic}")
                nc.vector.tensor_scalar(out=r1, in0=o, scalar1=0.0, scalar2=None, op0=ALU.max)
                (nc.gpsimd if GPSC else nc.vector).tensor_tensor(out=e1, in0=e1, in1=r1, op=ALU.add)
                nc.vector.tensor_scalar(out=e1, in0=e1, scalar1=1.0, scalar2=None,
                                        op0=ALU.subtract)

                # LayerNorm over 256 features
                st6 = fpool.tile([128, 6], f32, tag="st6", name=f"st6_{ic}")
                nc.vector.bn_stats(out=st6, in_=e1)
                mv = fpool.tile([128, 2], f32, tag="mv", name=f"mv{ic}")
                nc.vector.bn_aggr(out=mv, in_=st6)
                sd = fpool.tile([128, 1], f32, tag="sd", name=f"sd{ic}")
                nc.scalar.activation(out=sd, in_=mv[:, 1:2], func=AF.Sqrt, bias=eps_t)
                rstd = fpool.tile([128, 1], f32, tag="rstd", name=f"rstd{ic}")
                nc.vector.reciprocal(rstd, sd)
                xm = fpool.tile([128, 256], f32, tag="xm", name=f"xm{ic}")
                nc.vector.tensor_scalar(
                    out=xm, in0=e1, scalar1=mv[:, 0:1], scalar2=rstd,
                    op0=ALU.subtract, op1=ALU.mult,
                )
                (nc.gpsimd if GPSC else nc.vector).tensor_tensor(out=xm, in0=xm, in1=gb_sb[:, 0, :], op=ALU.mult)
                (nc.gpsimd if GPSC else nc.vector).tensor_tensor(out=xm, in0=xm, in1=gb_sb[:, 1, :], op=ALU.add)
                nc.scalar.dma_start(out=out[ic * 128:(ic + 1) * 128, :], in_=xm)

    nc.compile()
    return nc


def _prep_in_maps(x, adj, W, a, gamma, beta):
    x = np.asarray(x)
    adj = np.asarray(adj)
    W = np.asarray(W, np.float32)
    a = np.asarray(a, np.float32)
    gamma = np.asarray(gamma, np.float32)
    beta = np.asarray(beta, np.float32)

    # weight folding (host): w_src = W @ a[:, :D], w_dst = W @ a[:, D:]
    w_src = np.einsum("hqd,hd->hq", W, a[:, :D]).astype(np.float32)   # (H, Q)
    w_dst = np.einsum("hqd,hd->hq", W, a[:, D:]).astype(np.float32)   # (H, Q)
    Wp = np.concatenate([W, w_src[:, :, None], w_dst[:, :, None]], axis=2)  # (H, Q, 66)
    wp_in = np.ascontiguousarray(
        Wp.transpose(1, 0, 2).reshape(NQ, 128, H, 66)
    ).astype(ml_dtypes.bfloat16)

    xb = x.astype(ml_dtypes.bfloat16)
    xtb = np.ascontiguousarray(xb.T)                      # (Q, N)
    mbf = (adj > 0).astype(ml_dtypes.bfloat16)
    np.fill_diagonal(mbf, np.float32(1.0))
    mbt_full = np.ascontiguousarray(mbf.T)                # (N, N): mbt_full[j, i]
    gb_in = np.broadcast_to(
        np.stack([gamma, beta])[None, :, :], (128, 2, 256)
    ).astype(np.float32).copy()

    in_maps = []
    for c in range(NCORES):
        off = c * S
        in_maps.append({
            "xt": xtb,
            "xst": np.ascontiguousarray(xtb[:, off:off + S]),
            "mbt": np.ascontiguousarray(mbt_full[:, off:off + S]),
            "wp": wp_in,
            "gb": gb_in,
        })
    return in_maps


def kernel(x, adj, W, a, gamma, beta):
    in_maps = _prep_in_maps(x, adj, W, a, gamma, beta)

    key = ("gat", REPEAT, ACT_JC, GPS_JC, GPSC)
    if key not in _NC_CACHE:
        _NC_CACHE[key] = _build()
    nc = _NC_CACHE[key]

    trace = bool(int(os.environ.get("KERNEL_TRACE", "0")))
    try:
        import antenv.axon_hooks  # noqa: F401
    except Exception:
        trace = False
    res = run_bass_kernel_spmd(nc, in_maps, core_ids=list(range(NCORES)), trace=trace)
    if trace and res.exec_time_ns is not None:
        print(f"HW exec time: {res.exec_time_ns} ns")
        print(f"mean exec time: {res.mean_exec_time_ns} ns")
        if res.instructions_and_trace is not None:
            print("trace:", res.instructions_and_trace[1])
    return np.concatenate([res.results[c]["out"] for c in range(NCORES)], axis=0)


# revision 18
# speedup vs baseline: 1.5974x; 1.0225x over previous
"""Multi-head graph attention layer (GAT) on 8 TRN2 NeuronCores.

Row-parallel sharding: core c owns destination rows [c*512, (c+1)*512).
Scores are materialized transposed (source j on partitions, dest i on free dim)
so that alpha @ Wx is a single accumulating matmul per (j-chunk, head) with the
softmax denominator obtained from an appended ones-column in lhsT.

Score math: exp(leakyrelu(s)) with s = a_src[i] + a_dst[j] factors as
    exp(prelu(s)) = exp(0.2 s) * max(exp(0.8 s), 1)
The rank-1 factor exp(0.2 s) = exp(.2 a_src[i]) * exp(.2 a_dst[j]) needs no
elementwise work: the i-part is constant per softmax row and cancels between
numerator and denominator; the j-part folds into the matmul weights
(L2 = exp(.2 a_dst[j]) * [Wx | 1], built once in phase A). Per j-chunk the
loop computes  un' = max(exp(.8 s), 1) * mask  via one of two paths that
differ only in which engine produces exp(.8 s):
  PE+ACT path: rank-2 matmul builds .8 s in PSUM (hi/lo bf16 split keeps
               f32-ish accuracy), one batched Exp (PSUM->SBUF);
  DVE path:    exp(.8 s) = R8[i]*G8[j] as 4 tensor_scalar mults (4x mode).
Both finish with ONE batched scalar_tensor_tensor: (u max 1) * mask, where
mask broadcasts across heads with a stride-0 AP.
"""

import os
import numpy as np
import ml_dtypes

import concourse.bacc as bacc
import concourse.mybir as mybir
import concourse.tile as tile
from concourse.bass_utils import run_bass_kernel_spmd
from concourse.masks import make_identity

N, Q, D, H = 4096, 512, 64, 4
NCORES = 8
S = N // NCORES          # 512 dest rows per core
NJ = N // 128            # 32 j-chunks
NI = S // 128            # 4 i-chunks per core
NQ = Q // 128            # 4 q-chunks
NEG = 0.2
POS = 1.0 - NEG          # 0.8
LN_EPS = 1e-5
ACT_JC = int(os.environ.get("ACT_JC", "12"))   # of 32 jc groups on the PE+ACT path
GPS_JC = int(os.environ.get("GPS_JC", "0"))    # of 32: final stt on gpsimd
GPSC = bool(int(os.environ.get("GPSC", "1")))  # phase-C TTs on gpsimd
REPEAT = int(os.environ.get("REPEAT", "1"))    # repeat main loop (timing amplification)
TPOOL_B = int(os.environ.get("TPOOL_B", "3"))
MPOOL_B = int(os.environ.get("MPOOL_B", "16"))
f32 = mybir.dt.float32
bf16 = mybir.dt.bfloat16
AF = mybir.ActivationFunctionType
ALU = mybir.AluOpType

_NC_CACHE = {}


def _build():
    nc = bacc.Bacc("TRN2", target_bir_lowering=False)

    xt = nc.declare_dram_parameter("xt", [Q, N], bf16, isOutput=False)
    xst = nc.declare_dram_parameter("xst", [Q, S], bf16, isOutput=False)
    mbt = nc.declare_dram_parameter("mbt", [N, S], bf16, isOutput=False)
    wp = nc.declare_dram_parameter("wp", [NQ, 128, H, 66], bf16, isOutput=False)
    gb = nc.declare_dram_parameter("gb", [128, 2, 256], f32, isOutput=False)
    out = nc.declare_dram_parameter("out", [S, 256], f32, isOutput=True)

    with tile.TileContext(nc) as tc:
        with (
            tc.tile_pool(name="consts", bufs=1) as consts,
            tc.tile_pool(name="mpool", bufs=MPOOL_B) as mpool,
            tc.tile_pool(name="tpool", bufs=TPOOL_B) as tpool,
            tc.tile_pool(name="fpool", bufs=4) as fpool,
            tc.tile_pool(name="ppool", bufs=1, space="PSUM") as ppool,
        ):
            def ctile(shape, dtype, tg):
                return consts.tile(shape, dtype, tag=tg, name=tg)

            def ptile(name):
                # 2-bank PSUM scratch (tag-rotated, 2 deep); phase A/C carve
                # views out of it, the loop uses it for score tiles
                return ppool.tile([128, 2, 512], f32, tag="sc", name=name, bufs=2)

            # ---------------- constants / small inputs ----------------
            wp_sb = ctile([128, NQ, H, 66], bf16, "wp_sb")
            nc.scalar.dma_start(out=wp_sb, in_=wp.rearrange("qc p h d -> p qc h d"))
            gb_sb = ctile([128, 2, 256], f32, "gb_sb")
            nc.scalar.dma_start(out=gb_sb, in_=gb[:, :, :])
            ident = ctile([128, 128], f32, "ident")
            make_identity(nc, ident)

            eps_t = ctile([128, 1], f32, "eps_t")
            nc.vector.memset(eps_t, LN_EPS)

            # ---------------- xT loads (host pre-transposed) ----------------
            xsT_sb = ctile([128, NQ, S], bf16, "xsT_sb")
            nc.scalar.dma_start(out=xsT_sb, in_=xst.rearrange("(qc p) n -> p qc n", p=128))
            xt_sb = ctile([128, NQ, N], bf16, "xt_sb")
            for ch in range(8):
                n0, n1 = ch * (N // 8), (ch + 1) * (N // 8)
                nc.sync.dma_start(
                    out=xt_sb[:, :, n0:n1],
                    in_=xt[:, n0:n1].rearrange("(qc p) n -> p qc n", p=128),
                )

            # ---------------- phase A: Wx' = x @ [W | w_src | w_dst] ----------------
            # L2_sb[:, jc, h, 0:64] = exp(.2 a_dst[j]) * Wx (bf16),
            # col 64 = exp(.2 a_dst[j])  (denominator column)
            L2_sb = ctile([128, NJ, H, 65], bf16, "L2_sb")
            F2 = ctile([128, NJ, H], f32, "F2")     # exp(.2 a_dst)
            G8 = ctile([128, NJ, H], f32, "G8")     # exp(.8 a_dst)
            G4 = ctile([128, NJ, H], f32, "G4")
            ad_sb = ctile([128, NJ, H, 2], f32, "ad_sb")  # [...,0]=a_src(n) [...,1]=a_dst(n)
            pwt = None
            for nc_ in range(NJ):
                # two pw slots per 2-bank psum tile -> effective depth 4
                if nc_ % 2 == 0:
                    pwt = ptile(f"pw{nc_}")
                pw = pwt[:, nc_ % 2, :][:, 0:H * 66].rearrange(
                    "p (h d) -> p h d", d=66)
                for qc in range(NQ):
                    nc.tensor.matmul(
                        pw, xt_sb[:, qc, nc_ * 128:(nc_ + 1) * 128], wp_sb[:, qc, :, :],
                        start=(qc == 0), stop=(qc == NQ - 1),
                    )
                nc.vector.tensor_copy(ad_sb[:, nc_, :, :], pw[:, :, 64:66])
                nc.scalar.activation(out=F2[:, nc_, :], in_=ad_sb[:, nc_, :, 1],
                                     func=AF.Exp, scale=NEG)
                for h in range(H):
                    if h % 2 == 0:
                        nc.vector.tensor_scalar(
                            out=L2_sb[:, nc_, h, 0:64], in0=pw[:, h, 0:64],
                            scalar1=F2[:, nc_, h:h + 1], scalar2=None, op0=ALU.mult,
                        )
                    else:
                        nc.scalar.activation(
                            out=L2_sb[:, nc_, h, 0:64], in_=pw[:, h, 0:64],
                            func=AF.Copy, scale=F2[:, nc_, h:h + 1],
                        )
            # denominator column: L2[:, :, :, 64] = F2
            nc.vector.tensor_copy(L2_sb[:, :, :, 64], F2)
            # G8 = F2^4 = exp(.8 a_dst)
            nc.scalar.activation(out=G4, in_=F2, func=AF.Square)
            nc.scalar.activation(out=G8, in_=G4, func=AF.Square)

            # ad8 = .8 a_dst (per-partition bias rows for the ACT path)
            ad8 = ctile([128, NJ, H], f32, "ad8")
            nc.vector.tensor_scalar(out=ad8, in0=ad_sb[:, :, :, 1], scalar1=POS,
                                    scalar2=None, op0=ALU.mult)

            # ---------------- a_src rows for this core's shard ----------------
            pat = ptile("p_asrc")
            p_asrc = pat.rearrange("p a b -> p (a b)")[:, 0:512]
            for qc in range(NQ):
                nc.tensor.matmul(
                    p_asrc[0:H, :], wp_sb[:, qc, :, 64], xsT_sb[:, qc, :],
                    start=(qc == 0), stop=(qc == NQ - 1),
                )
            asrc_row = ctile([H, S], f32, "asrc_row")
            nc.vector.tensor_copy(asrc_row, p_asrc[0:H, :])
            r8_row = ctile([H, S], bf16, "r8_row")
            nc.scalar.activation(out=r8_row, in_=asrc_row, func=AF.Exp, scale=POS)

            # broadcast a_src / exp(.8 a_src) row h across partitions via
            # selector matmuls (sel[:, h, :] has ones on partition h only)
            iota_p128 = ctile([128, 128], f32, "iota_p128")
            nc.gpsimd.iota(iota_p128, pattern=[[0, 128]], base=0, channel_multiplier=1,
                           allow_small_or_imprecise_dtypes=True)
            sel_t = ctile([128, H, 128], f32, "sel_t")
            sel_tb = ctile([128, H, 128], bf16, "sel_tb")
            for h in range(H):
                nc.vector.tensor_scalar(
                    out=sel_t[:, h, :], in0=iota_p128, scalar1=float(h), scalar2=None,
                    op0=ALU.is_equal,
                )
                nc.vector.tensor_scalar(
                    out=sel_tb[:, h, :], in0=iota_p128, scalar1=float(h), scalar2=None,
                    op0=ALU.is_equal,
                )
            asrc_b = ctile([128, H, S], f32, "asrc_b")   # a_src broadcast (ACT path)
            R8b = ctile([128, H, S], bf16, "R8b")        # exp(.8 a_src) broadcast
            for h in range(H):
                pbt = ptile(f"pb_a{h}")
                pb = pbt.rearrange("p a b -> p (a b)")[:, 0:512]
                nc.tensor.matmul(pb, sel_t[0:H, h, :], asrc_row, start=True, stop=True)
                nc.vector.tensor_copy(asrc_b[:, h, :], pb)
                pbt = ptile(f"pb_r8{h}")
                pb = pbt.rearrange("p a b -> p (a b)")[:, 0:512]
                nc.tensor.matmul(pb, sel_tb[0:H, h, :], r8_row, start=True, stop=True)
                nc.scalar.copy(R8b[:, h, :], pb)

            # ---------------- phase B: attention main loop ----------------
            # psum accumulators, one [65, 512] bank per head:
            # rows 0:64 = outT'[d, i] (unnormalized); row 64 = S'[i] (denominator)
            poT = [ppool.tile([65, 512], f32, tag=f"oT{h}", name=f"oT{h}")
                   for h in range(H)]

            import contextlib
            loop_cm = tc.For_i(0, REPEAT, 1) if REPEAT > 1 else contextlib.nullcontext()
            with loop_cm:
              rep = 0
              for jc in range(NJ):
                mT = mpool.tile([128, S], bf16, tag="mask", name=f"mT{rep}_{jc}")
                # scalar ring: avoids queueing behind the 4MB xt load on sync
                nc.scalar.dma_start(out=mT, in_=mbt[jc * 128:(jc + 1) * 128, :])

                if (jc * ACT_JC) % NJ < ACT_JC:
                    # ACT path: t = Relu(.8 s), u = Exp(t) = max(exp(.8 s), 1)
                    t = tpool.tile([128, H, S], f32, tag="t", name=f"t{rep}_{jc}",
                                   bufs=2)
                    for h in range(H):
                        nc.scalar.activation(
                            out=t[:, h, :], in_=asrc_b[:, h, :], func=AF.Relu,
                            bias=ad8[:, jc, h:h + 1], scale=POS,
                        )
                    src = tpool.tile([128, H, S], bf16, tag="u", name=f"u{rep}_{jc}")
                    nc.scalar.activation(out=src, in_=t, func=AF.Exp)
                else:
                    # DVE path: u = max(R8[i] * G8[j], 1)
                    src = tpool.tile([128, H, S], bf16, tag="m", name=f"m{rep}_{jc}")
                    for h in range(H):
                        nc.vector.tensor_scalar(
                            out=src[:, h, :], in0=R8b[:, h, :],
                            scalar1=G8[:, jc, h:h + 1], scalar2=1.0,
                            op0=ALU.mult, op1=ALU.max,
                        )

                # un = u * mask  (mask broadcast over h with a stride-0 AP)
                un = tpool.tile([128, H, S], bf16, tag="un", name=f"un{rep}_{jc}")
                mTb = mT.unsqueeze(1).broadcast_to([128, H, S])
                eng = nc.gpsimd if (jc * GPS_JC) % NJ < GPS_JC else nc.vector
                eng.tensor_tensor(out=un, in0=src, in1=mTb, op=ALU.mult)

                for h in range(H):
                    nc.tensor.matmul(
                        poT[h], L2_sb[:, jc, h, 0:65], un[:, h, :],
                        start=(jc == 0), stop=(jc == NJ - 1),
                    )

            # ---------------- phase C: normalize, ELU, LayerNorm ----------------
            oT_sb = ctile([65, H, S], f32, "oT_sb")
            for h in range(H):
                if h % 2 == 0:
                    nc.vector.tensor_copy(oT_sb[:, h, :], poT[h])
                else:
                    nc.scalar.copy(oT_sb[:, h, :], poT[h])

            for ic in range(NI):
                p2t = ptile(f"p2_{ic}")
                p2 = p2t.rearrange("p a b -> p (a b)")[:, 0:H * 66].rearrange(
                    "p (h d) -> p h d", d=66)
                for h in range(H):
                    nc.tensor.transpose(
                        p2[:, h, 0:65],
                        oT_sb[:, h, ic * 128:(ic + 1) * 128],
                        ident[0:65, 0:65],
                    )
                s_sb = fpool.tile([128, H], f32, tag="s", name=f"s{ic}")
                nc.vector.tensor_copy(s_sb, p2[:, :, 64])
                rs = fpool.tile([128, H], f32, tag="rs", name=f"rs{ic}")
                nc.vector.reciprocal(rs, s_sb)

                o = fpool.tile([128, 256], f32, tag="o", name=f"o{ic}")
                ov = o.rearrange("p (h d) -> p h d", h=H)
                for h in range(H):
                    nc.vector.tensor_scalar(
                        out=ov[:, h, :], in0=p2[:, h, 0:64], scalar1=rs[:, h:h + 1],
                        scalar2=None, op0=ALU.mult,
                    )
                # ELU: exp(min(o,0)) + max(o,0) - 1
                m1 = fpool.tile([128, 256], f32, tag="m1", name=f"m1_{ic}")
                nc.vector.tensor_scalar(out=m1, in0=o, scalar1=0.0, scalar2=None, op0=ALU.min)
                e1 = fpool.tile([128, 256], f32, tag="e1", name=f"e1_{ic}")
                nc.scalar.activation(out=e1, in_=m1, func=AF.Exp)
                r1 = fpool.tile([128, 256], f32, tag="r1", name=f"r1_{ic}")
                nc.vector.tensor_scalar(out=r1, in0=o, scalar1=0.0, scalar2=None, op0=ALU.max)
                (nc.gpsimd if GPSC else nc.vector).tensor_tensor(out=e1, in0=e1, in1=r1, op=ALU.add)
                nc.vector.tensor_scalar(out=e1, in0=e1, scalar1=1.0, scalar2=None,
                                        op0=ALU.subtract)

                # LayerNorm over 256 features
                st6 = fpool.tile([128, 6], f32, tag="st6", name=f"st6_{ic}")
                nc.vector.bn_stats(out=st6, in_=e1)
                mv = fpool.tile([128, 2], f32, tag="mv", name=f"mv{ic}")
                nc.vector.bn_aggr(out=mv, in_=st6)
                sd = fpool.tile([128, 1], f32, tag="sd", name=f"sd{ic}")
                nc.scalar.activation(out=sd, in_=mv[:, 1:2], func=AF.Sqrt, bias=eps_t)
                rstd = fpool.tile([128, 1], f32, tag="rstd", name=f"rstd{ic}")
                nc.vector.reciprocal(rstd, sd)
                xm = fpool.tile([128, 256], f32, tag="xm", name=f"xm{ic}")
                nc.vector.tensor_scalar(
                    out=xm, in0=e1, scalar1=mv[:, 0:1], scalar2=rstd,
                    op0=ALU.subtract, op1=ALU.mult,
                )
                (nc.gpsimd if GPSC else nc.vector).tensor_tensor(out=xm, in0=xm, in1=gb_sb[:, 0, :], op=ALU.mult)
                (nc.gpsimd if GPSC else nc.vector).tensor_tensor(out=xm, in0=xm, in1=gb_sb[:, 1, :], op=ALU.add)
                nc.scalar.dma_start(out=out[ic * 128:(ic + 1) * 128, :], in_=xm)

    nc.compile()
    return nc


def _prep_in_maps(x, adj, W, a, gamma, beta):
    x = np.asarray(x)
    adj = np.asarray(adj)
    W = np.asarray(W, np.float32)
    a = np.asarray(a, np.float32)
    gamma = np.asarray(gamma, np.float32)
    beta = np.asarray(beta, np.float32)

    # weight folding (host): w_src = W @ a[:, :D], w_dst = W @ a[:, D:]
    w_src = np.einsum("hqd,hd->hq", W, a[:, :D]).astype(np.float32)   # (H, Q)
    w_dst = np.einsum("hqd,hd->hq", W, a[:, D:]).astype(np.float32)   # (H, Q)
    Wp = np.concatenate([W, w_src[:, :, None], w_dst[:, :, None]], axis=2)  # (H, Q, 66)
    wp_in = np.ascontiguousarray(
        Wp.transpose(1, 0, 2).reshape(NQ, 128, H, 66)
    ).astype(ml_dtypes.bfloat16)

    xb = x.astype(ml_dtypes.bfloat16)
    xtb = np.ascontiguousarray(xb.T)                      # (Q, N)
    mbf = (adj > 0).astype(ml_dtypes.bfloat16)
    np.fill_diagonal(mbf, np.float32(1.0))
    mbt_full = np.ascontiguousarray(mbf.T)                # (N, N): mbt_full[j, i]
    gb_in = np.broadcast_to(
        np.stack([gamma, beta])[None, :, :], (128, 2, 256)
    ).astype(np.float32).copy()

    in_maps = []
    for c in range(NCORES):
        off = c * S
        in_maps.append({
            "xt": xtb,
            "xst": np.ascontiguousarray(xtb[:, off:off + S]),
            "mbt": np.ascontiguousarray(mbt_full[:, off:off + S]),
            "wp": wp_in,
            "gb": gb_in,
        })
    return in_maps


def kernel(x, adj, W, a, gamma, beta):
    in_maps = _prep_in_maps(x, adj, W, a, gamma, beta)

    key = ("gat", REPEAT, ACT_JC, GPS_JC, GPSC)
    if key not in _NC_CACHE:
        _NC_CACHE[key] = _build()
    nc = _NC_CACHE[key]

    trace = bool(int(os.environ.get("KERNEL_TRACE", "0")))
    try:
        import antenv.axon_hooks  # noqa: F401
    except Exception:
        trace = False
    res = run_bass_kernel_spmd(nc, in_maps, core_ids=list(range(NCORES)), trace=trace)
    if trace and res.exec_time_ns is not None:
        print(f"HW exec time: {res.exec_time_ns} ns")
        print(f"mean exec time: {res.mean_exec_time_ns} ns")
        if res.instructions_and_trace is not None:
            print("trace:", res.instructions_and_trace[1])
    return np.concatenate([res.results[c]["out"] for c in range(NCORES)], axis=0)


# revision 20
# speedup vs baseline: 1.6382x; 1.0256x over previous
"""Multi-head graph attention layer (GAT) on 8 TRN2 NeuronCores.

Row-parallel sharding: core c owns destination rows [c*512, (c+1)*512).
Scores are materialized transposed (source j on partitions, dest i on free dim)
so that alpha @ Wx is a single accumulating matmul per (j-chunk, head) with the
softmax denominator obtained from an appended ones-column in lhsT.

Score math: exp(leakyrelu(s)) with s = a_src[i] + a_dst[j] factors as
    exp(prelu(s)) = exp(0.2 s) * max(exp(0.8 s), 1)
The rank-1 factor exp(0.2 s) = exp(.2 a_src[i]) * exp(.2 a_dst[j]) needs no
elementwise work: the i-part is constant per softmax row and cancels between
numerator and denominator; the j-part folds into the matmul weights
(L2 = exp(.2 a_dst[j]) * [Wx | 1], built once in phase A). Per j-chunk the
loop computes  un' = max(exp(.8 s), 1) * mask  via one of two paths that
differ only in which engine produces max(exp(.8 s), 1):
  ACT path: Relu(.8 a_src + .8 a_dst[j] bias) then Exp (head-batched);
  DVE path: tensor_scalar (R8[i]*G8[j] then max 1) per head.
Both finish with ONE head-batched tensor_tensor multiply against the mask,
which broadcasts across heads with a stride-0 AP. ACT_JC tunes the path
split so ACT and DVE engine loads balance; gpsimd offload was measured and
hurts (shared SBUF port), so the loop leaves it idle.

For the single-exec (REPEAT==1) build, phase A is software-pipelined into
the attention loop (the loop trails by one 8-chunk group) so Wx/L2
production overlaps attention consumption instead of draining first.
"""

import os
import numpy as np
import ml_dtypes

import concourse.bacc as bacc
import concourse.mybir as mybir
import concourse.tile as tile
from concourse.bass_utils import run_bass_kernel_spmd
from concourse.masks import make_identity

N, Q, D, H = 4096, 512, 64, 4
NCORES = 8
S = N // NCORES          # 512 dest rows per core
NJ = N // 128            # 32 j-chunks
NI = S // 128            # 4 i-chunks per core
NQ = Q // 128            # 4 q-chunks
NEG = 0.2
POS = 1.0 - NEG          # 0.8
LN_EPS = 1e-5
ACT_JC = int(os.environ.get("ACT_JC", "12"))   # of 32 jc groups on the PE+ACT path
GPS_JC = int(os.environ.get("GPS_JC", "0"))    # of 32: final stt on gpsimd
GPSC = bool(int(os.environ.get("GPSC", "1")))  # phase-C TTs on gpsimd
REPEAT = int(os.environ.get("REPEAT", "1"))    # repeat main loop (timing amplification)
TPOOL_B = int(os.environ.get("TPOOL_B", "3"))
MPOOL_B = int(os.environ.get("MPOOL_B", "16"))
f32 = mybir.dt.float32
bf16 = mybir.dt.bfloat16
AF = mybir.ActivationFunctionType
ALU = mybir.AluOpType

_NC_CACHE = {}


def _build():
    nc = bacc.Bacc("TRN2", target_bir_lowering=False)

    xt = nc.declare_dram_parameter("xt", [Q, N], bf16, isOutput=False)
    xst = nc.declare_dram_parameter("xst", [Q, S], bf16, isOutput=False)
    mbt = nc.declare_dram_parameter("mbt", [N, S], bf16, isOutput=False)
    wp = nc.declare_dram_parameter("wp", [NQ, 128, H, 66], bf16, isOutput=False)
    gb = nc.declare_dram_parameter("gb", [128, 2, 256], f32, isOutput=False)
    out = nc.declare_dram_parameter("out", [S, 256], f32, isOutput=True)

    with tile.TileContext(nc) as tc:
        with (
            tc.tile_pool(name="consts", bufs=1) as consts,
            tc.tile_pool(name="mpool", bufs=MPOOL_B) as mpool,
            tc.tile_pool(name="tpool", bufs=TPOOL_B) as tpool,
            tc.tile_pool(name="fpool", bufs=4) as fpool,
            tc.tile_pool(name="ppool", bufs=1, space="PSUM") as ppool,
        ):
            def ctile(shape, dtype, tg):
                return consts.tile(shape, dtype, tag=tg, name=tg)

            def ptile(name):
                # 2-bank PSUM scratch (tag-rotated, 2 deep); phase A/C carve
                # views out of it, the loop uses it for score tiles
                return ppool.tile([128, 2, 512], f32, tag="sc", name=name, bufs=2)

            # ---------------- constants / small inputs ----------------
            wp_sb = ctile([128, NQ, H, 66], bf16, "wp_sb")
            nc.scalar.dma_start(out=wp_sb, in_=wp.rearrange("qc p h d -> p qc h d"))
            gb_sb = ctile([128, 2, 256], f32, "gb_sb")
            nc.scalar.dma_start(out=gb_sb, in_=gb[:, :, :])
            ident = ctile([128, 128], f32, "ident")
            make_identity(nc, ident)

            eps_t = ctile([128, 1], f32, "eps_t")
            nc.vector.memset(eps_t, LN_EPS)

            # ---------------- xT loads (host pre-transposed) ----------------
            xsT_sb = ctile([128, NQ, S], bf16, "xsT_sb")
            nc.scalar.dma_start(out=xsT_sb, in_=xst.rearrange("(qc p) n -> p qc n", p=128))
            xt_sb = ctile([128, NQ, N], bf16, "xt_sb")
            for ch in range(8):
                n0, n1 = ch * (N // 8), (ch + 1) * (N // 8)
                nc.sync.dma_start(
                    out=xt_sb[:, :, n0:n1],
                    in_=xt[:, n0:n1].rearrange("(qc p) n -> p qc n", p=128),
                )

            # ---------------- a_src rows + broadcasts (needed before the loop) ----
            pat = ptile("p_asrc")
            p_asrc = pat.rearrange("p a b -> p (a b)")[:, 0:512]
            for qc in range(NQ):
                nc.tensor.matmul(
                    p_asrc[0:H, :], wp_sb[:, qc, :, 64], xsT_sb[:, qc, :],
                    start=(qc == 0), stop=(qc == NQ - 1),
                )
            asrc_row = ctile([H, S], f32, "asrc_row")
            nc.vector.tensor_copy(asrc_row, p_asrc[0:H, :])
            r8_row = ctile([H, S], bf16, "r8_row")
            nc.scalar.activation(out=r8_row, in_=asrc_row, func=AF.Exp, scale=POS)

            # broadcast a_src / exp(.8 a_src) row h across partitions via
            # selector matmuls (sel[:, h, :] has ones on partition h only)
            iota_p128 = ctile([128, 128], f32, "iota_p128")
            nc.gpsimd.iota(iota_p128, pattern=[[0, 128]], base=0, channel_multiplier=1,
                           allow_small_or_imprecise_dtypes=True)
            sel_t = ctile([128, H, 128], f32, "sel_t")
            sel_tb = ctile([128, H, 128], bf16, "sel_tb")
            for h in range(H):
                nc.vector.tensor_scalar(
                    out=sel_t[:, h, :], in0=iota_p128, scalar1=float(h), scalar2=None,
                    op0=ALU.is_equal,
                )
                nc.vector.tensor_scalar(
                    out=sel_tb[:, h, :], in0=iota_p128, scalar1=float(h), scalar2=None,
                    op0=ALU.is_equal,
                )
            asrc_b = ctile([128, H, S], f32, "asrc_b")   # a_src broadcast (ACT path)
            R8b = ctile([128, H, S], bf16, "R8b")        # exp(.8 a_src) broadcast
            for h in range(H):
                pbt = ptile(f"pb_a{h}")
                pb = pbt.rearrange("p a b -> p (a b)")[:, 0:512]
                nc.tensor.matmul(pb, sel_t[0:H, h, :], asrc_row, start=True, stop=True)
                nc.vector.tensor_copy(asrc_b[:, h, :], pb)
                pbt = ptile(f"pb_r8{h}")
                pb = pbt.rearrange("p a b -> p (a b)")[:, 0:512]
                nc.tensor.matmul(pb, sel_tb[0:H, h, :], r8_row, start=True, stop=True)
                nc.scalar.copy(R8b[:, h, :], pb)

            # ---------------- phase A: Wx' = x @ [W | w_src | w_dst] ----------------
            # L2_sb[:, jc, h, 0:64] = exp(.2 a_dst[j]) * Wx (bf16),
            # col 64 = exp(.2 a_dst[j])  (denominator column)
            L2_sb = ctile([128, NJ, H, 65], bf16, "L2_sb")
            F2 = ctile([128, NJ, H], f32, "F2")     # exp(.2 a_dst)
            G8 = ctile([128, NJ, H], f32, "G8")     # exp(.8 a_dst)
            ad8 = ctile([128, NJ, H], f32, "ad8")   # .8 a_dst (ACT-path bias)
            ad_sb = ctile([128, NJ, H, 2], f32, "ad_sb")  # [...,0]=a_src(n) [...,1]=a_dst(n)
            pw_slots = [None]

            def phase_a_step(nc_):
                # two pw slots per 2-bank psum tile -> effective depth 4
                if nc_ % 2 == 0:
                    pw_slots[0] = ptile(f"pw{nc_}")
                pw = pw_slots[0][:, nc_ % 2, :][:, 0:H * 66].rearrange(
                    "p (h d) -> p h d", d=66)
                for qc in range(NQ):
                    nc.tensor.matmul(
                        pw, xt_sb[:, qc, nc_ * 128:(nc_ + 1) * 128], wp_sb[:, qc, :, :],
                        start=(qc == 0), stop=(qc == NQ - 1),
                    )
                nc.vector.tensor_copy(ad_sb[:, nc_, :, :], pw[:, :, 64:66])
                nc.scalar.activation(out=F2[:, nc_, :], in_=ad_sb[:, nc_, :, 1],
                                     func=AF.Exp, scale=NEG)
                for h in range(H):
                    if h % 2 == 0:
                        nc.vector.tensor_scalar(
                            out=L2_sb[:, nc_, h, 0:64], in0=pw[:, h, 0:64],
                            scalar1=F2[:, nc_, h:h + 1], scalar2=None, op0=ALU.mult,
                        )
                    else:
                        nc.scalar.activation(
                            out=L2_sb[:, nc_, h, 0:64], in_=pw[:, h, 0:64],
                            func=AF.Copy, scale=F2[:, nc_, h:h + 1],
                        )
                if nc_ % 8 == 7:
                    g0 = nc_ - 7
                    # denominator column + loop-side per-j factors for the group
                    nc.vector.tensor_copy(L2_sb[:, g0:nc_ + 1, :, 64],
                                          F2[:, g0:nc_ + 1, :])
                    nc.scalar.activation(out=G8[:, g0:nc_ + 1, :],
                                         in_=ad_sb[:, g0:nc_ + 1, :, 1],
                                         func=AF.Exp, scale=POS)
                    nc.vector.tensor_scalar(out=ad8[:, g0:nc_ + 1, :],
                                            in0=ad_sb[:, g0:nc_ + 1, :, 1],
                                            scalar1=POS, scalar2=None, op0=ALU.mult)

            # ---------------- phase B: attention main loop ----------------
            # psum accumulators, one [65, 512] bank per head:
            # rows 0:64 = outT'[d, i] (unnormalized); row 64 = S'[i] (denominator)
            poT = [ppool.tile([65, 512], f32, tag=f"oT{h}", name=f"oT{h}")
                   for h in range(H)]

            def loop_body(jc, rep):
                mT = mpool.tile([128, S], bf16, tag="mask", name=f"mT{rep}_{jc}")
                # scalar ring: avoids queueing behind the 4MB xt load on sync
                nc.scalar.dma_start(out=mT, in_=mbt[jc * 128:(jc + 1) * 128, :])

                if (jc * ACT_JC) % NJ < ACT_JC:
                    # ACT path: t = Relu(.8 s), u = Exp(t) = max(exp(.8 s), 1)
                    t = tpool.tile([128, H, S], f32, tag="t", name=f"t{rep}_{jc}",
                                   bufs=2)
                    for h in range(H):
                        nc.scalar.activation(
                            out=t[:, h, :], in_=asrc_b[:, h, :], func=AF.Relu,
                            bias=ad8[:, jc, h:h + 1], scale=POS,
                        )
                    src = tpool.tile([128, H, S], bf16, tag="u", name=f"u{rep}_{jc}")
                    nc.scalar.activation(out=src, in_=t, func=AF.Exp)
                else:
                    # DVE path: u = max(R8[i] * G8[j], 1)
                    src = tpool.tile([128, H, S], bf16, tag="m", name=f"m{rep}_{jc}")
                    for h in range(H):
                        nc.vector.tensor_scalar(
                            out=src[:, h, :], in0=R8b[:, h, :],
                            scalar1=G8[:, jc, h:h + 1], scalar2=1.0,
                            op0=ALU.mult, op1=ALU.max,
                        )

                # un = u * mask  (mask broadcast over h with a stride-0 AP)
                un = tpool.tile([128, H, S], bf16, tag="un", name=f"un{rep}_{jc}")
                mTb = mT.unsqueeze(1).broadcast_to([128, H, S])
                eng = nc.gpsimd if (jc * GPS_JC) % NJ < GPS_JC else nc.vector
                eng.tensor_tensor(out=un, in0=src, in1=mTb, op=ALU.mult)

                for h in range(H):
                    nc.tensor.matmul(
                        poT[h], L2_sb[:, jc, h, 0:65], un[:, h, :],
                        start=(jc == 0), stop=(jc == NJ - 1),
                    )

            if REPEAT == 1:
                # software-pipeline phase A into the loop: the loop trails
                # phase A by one 8-chunk group, so Wx/L2 production overlaps
                # attention consumption instead of draining first
                LAG = 8
                for k in range(NJ + LAG):
                    if k < NJ:
                        phase_a_step(k)
                    if k >= LAG:
                        loop_body(k - LAG, 0)
            else:
                for nc_ in range(NJ):
                    phase_a_step(nc_)
                with tc.For_i(0, REPEAT, 1):
                    for jc in range(NJ):
                        loop_body(jc, 0)

            # ---------------- phase C: normalize, ELU, LayerNorm ----------------
            oT_sb = ctile([65, H, S], f32, "oT_sb")
            for h in range(H):
                if h % 2 == 0:
                    nc.vector.tensor_copy(oT_sb[:, h, :], poT[h])
                else:
                    nc.scalar.copy(oT_sb[:, h, :], poT[h])

            for ic in range(NI):
                p2t = ptile(f"p2_{ic}")
                p2 = p2t.rearrange("p a b -> p (a b)")[:, 0:H * 66].rearrange(
                    "p (h d) -> p h d", d=66)
                for h in range(H):
                    nc.tensor.transpose(
                        p2[:, h, 0:65],
                        oT_sb[:, h, ic * 128:(ic + 1) * 128],
                        ident[0:65, 0:65],
                    )
                s_sb = fpool.tile([128, H], f32, tag="s", name=f"s{ic}")
                nc.vector.tensor_copy(s_sb, p2[:, :, 64])
                rs = fpool.tile([128, H], f32, tag="rs", name=f"rs{ic}")
                nc.vector.reciprocal(rs, s_sb)

                o = fpool.tile([128, 256], f32, tag="o", name=f"o{ic}")
                ov = o.rearrange("p (h d) -> p h d", h=H)
                for h in range(H):
                    nc.vector.tensor_scalar(
                        out=ov[:, h, :], in0=p2[:, h, 0:64], scalar1=rs[:, h:h + 1],
                        scalar2=None, op0=ALU.mult,
                    )
                # ELU: exp(min(o,0)) + max(o,0) - 1
                m1 = fpool.tile([128, 256], f32, tag="m1", name=f"m1_{ic}")
                nc.vector.tensor_scalar(out=m1, in0=o, scalar1=0.0, scalar2=None, op0=ALU.min)
                e1 = fpool.tile([128, 256], f32, tag="e1", name=f"e1_{ic}")
                nc.scalar.activation(out=e1, in_=m1, func=AF.Exp)
                r1 = fpool.tile([128, 256], f32, tag="r1", name=f"r1_{ic}")
                nc.vector.tensor_scalar(out=r1, in0=o, scalar1=0.0, scalar2=None, op0=ALU.max)
                (nc.gpsimd if GPSC else nc.vector).tensor_tensor(out=e1, in0=e1, in1=r1, op=ALU.add)
                nc.vector.tensor_scalar(out=e1, in0=e1, scalar1=1.0, scalar2=None,
                                        op0=ALU.subtract)

                # LayerNorm over 256 features
                st6 = fpool.tile([128, 6], f32, tag="st6", name=f"st6_{ic}")
                nc.vector.bn_stats(out=st6, in_=e1)
                mv = fpool.tile([128, 2], f32, tag="mv", name=f"mv{ic}")
                nc.vector.bn_aggr(out=mv, in_=st6)
                sd = fpool.tile([128, 1], f32, tag="sd", name=f"sd{ic}")
                nc.scalar.activation(out=sd, in_=mv[:, 1:2], func=AF.Sqrt, bias=eps_t)
                rstd = fpool.tile([128, 1], f32, tag="rstd", name=f"rstd{ic}")
                nc.vector.reciprocal(rstd, sd)
                xm = fpool.tile([128, 256], f32, tag="xm", name=f"xm{ic}")
                nc.vector.tensor_scalar(
                    out=xm, in0=e1, scalar1=mv[:, 0:1], scalar2=rstd,
                    op0=ALU.subtract, op1=ALU.mult,
                )
                (nc.gpsimd if GPSC else nc.vector).tensor_tensor(out=xm, in0=xm, in1=gb_sb[:, 0, :], op=ALU.mult)
                (nc.gpsimd if GPSC else nc.vector).tensor_tensor(out=xm, in0=xm, in1=gb_sb[:, 1, :], op=ALU.add)
                nc.scalar.dma_start(out=out[ic * 128:(ic + 1) * 128, :], in_=xm)

    nc.compile()
    return nc


def _prep_in_maps(x, adj, W, a, gamma, beta):
    x = np.asarray(x)
    adj = np.asarray(adj)
    W = np.asarray(W, np.float32)
    a = np.asarray(a, np.float32)
    gamma = np.asarray(gamma, np.float32)
    beta = np.asarray(beta, np.float32)

    # weight folding (host): w_src = W @ a[:, :D], w_dst = W @ a[:, D:]
    w_src = np.einsum("hqd,hd->hq", W, a[:, :D]).astype(np.float32)   # (H, Q)
    w_dst = np.einsum("hqd,hd->hq", W, a[:, D:]).astype(np.float32)   # (H, Q)
    Wp = np.concatenate([W, w_src[:, :, None], w_dst[:, :, None]], axis=2)  # (H, Q, 66)
    wp_in = np.ascontiguousarray(
        Wp.transpose(1, 0, 2).reshape(NQ, 128, H, 66)
    ).astype(ml_dtypes.bfloat16)

    xb = x.astype(ml_dtypes.bfloat16)
    xtb = np.ascontiguousarray(xb.T)                      # (Q, N)
    mbf = (adj > 0).astype(ml_dtypes.bfloat16)
    np.fill_diagonal(mbf, np.float32(1.0))
    mbt_full = np.ascontiguousarray(mbf.T)                # (N, N): mbt_full[j, i]
    gb_in = np.broadcast_to(
        np.stack([gamma, beta])[None, :, :], (128, 2, 256)
    ).astype(np.float32).copy()

    in_maps = []
    for c in range(NCORES):
        off = c * S
        in_maps.append({
            "xt": xtb,
            "xst": np.ascontiguousarray(xtb[:, off:off + S]),
            "mbt": np.ascontiguousarray(mbt_full[:, off:off + S]),
            "wp": wp_in,
            "gb": gb_in,
        })
    return in_maps


def kernel(x, adj, W, a, gamma, beta):
    in_maps = _prep_in_maps(x, adj, W, a, gamma, beta)

    key = ("gat", REPEAT, ACT_JC, GPS_JC, GPSC)
    if key not in _NC_CACHE:
        _NC_CACHE[key] = _build()
    nc = _NC_CACHE[key]

    trace = bool(int(os.environ.get("KERNEL_TRACE", "0")))
    try:
        import antenv.axon_hooks  # noqa: F401
    except Exception:
        trace = False
    res = run_bass_kernel_spmd(nc, in_maps, core_ids=list(range(NCORES)), trace=trace)
    if trace and res.exec_time_ns is not None:
        print(f"HW exec time: {res.exec_time_ns} ns")
        print(f"mean exec time: {res.mean_exec_time_ns} ns")
        if res.instructions_and_trace is not None:
            print("trace:", res.instructions_and_trace[1])
    return np.concatenate([res.results[c]["out"] for c in range(NCORES)], axis=0)
